# revision 9
# baseline (speedup 1.0000x reference)
"""Trainium2 Bass kernel for the KB criterion loss.

Math
----
reference:
    diff[b,i,j] = probs[b,j] - probs[b,i]
    loss = sum_ij mean_b (diff^2 * C[i,j]) / (n_pos + 1e-8),  n_pos = count(C > 0)

Expanding the square removes the [B,N,N] intermediate entirely:

    sum_b (P[b,i] - P[b,j])^2 = S2_i + S2_j - 2*G_ij
        with S2_j = sum_b P[b,j]^2   and   G = P^T P  (Gram matrix)

so   total = sum_ij C_ij * (S2_i + S2_j - 2 G_ij)
     loss  = (total / B) / (n_pos + 1e-8)

Sharding (8 cores)
------------------
Shard C by rows: core k owns rows S_k = [128k, 128k+128). P is replicated.
Each core moves ~0.5MB (bf16 P + bf16 C shard) vs 4MB of C with
batch-parallel sharding. Inputs are column-rolled by 128k so every core
runs the same program with its own row block mapped to local columns
[0:128). The transposed local P block (pure layout prep, like the roll)
ships as a third input so S2_i is available as a per-partition column.

Per-core pipeline (engines in parallel; all operands bf16, accum fp32):
  DVE : psq = P*P (two 512-col halves)
  PE  : per 512-col PSUM bank: d = P_Sk^T @ P  (Gram block, K=128 b)
                              + (-1/2 const)^T @ psq  (= -S2_j/2 bcast)
  ACT : s2i = rowsum(PT_Sk^2)  (Square + accum)  -> S2_i column
  DVE : (C * -2) * d  summed per partition  -> red[:,0:2]   (fused STT)
  ACT : C * s2i  summed per partition       -> red[:,2]  (Copy, scale AP)
  ACT : Sign(C)  summed per partition       -> red[:,3]  (n_pos count)
  Pool: partition-reduce red [128,4] -> [1,4], DMA out.

Host sums the 8 partial quadruples (the scalar all-reduce) and finishes
the division. bf16 operand rounding is well inside the 2e-2 tolerance
(errors are random over 2^20 summed terms; observed ~1e-3).
"""

import ml_dtypes
import numpy as np

import concourse.bass as bass
import concourse.tile as tile
from concourse import mybir
from concourse.bass_utils import run_bass_kernel_spmd

B = 128
N = 1024
NCORES = 8
SH = N // NCORES  # 128 rows of C per core
F32 = mybir.dt.float32
BF16 = mybir.dt.bfloat16
HALF = 512  # PSUM bank width in fp32
BF16NP = ml_dtypes.bfloat16


def build_bass() -> bass.Bass:
    nc = bass.Bass()
    p_d = nc.dram_tensor("probs_r", [B, N], BF16, kind="ExternalInput")
    pt_d = nc.dram_tensor("pt_r", [SH, B], BF16, kind="ExternalInput")
    c_d = nc.dram_tensor("co_r", [SH, N], BF16, kind="ExternalInput")
    o_d = nc.dram_tensor("out", [1, 4], F32, kind="ExternalOutput")

    with tile.TileContext(nc) as tc:
        with (
            tc.tile_pool(name="sb", bufs=1) as sb,
            tc.tile_pool(name="ps", bufs=1, space="PSUM") as ps,
        ):
            p_sb = sb.tile([B, N], BF16)
            pt_sb = sb.tile([SH, B], BF16)
            c_sb = sb.tile([SH, N], BF16)
            psq = sb.tile([B, N], BF16)
            nh = sb.tile([B, SH], BF16)  # const -1/2, lhsT of the S2_j matmul
            ptsq = sb.tile([SH, B], BF16)  # dead store of the Square pass
            s2i = sb.tile([SH, 1], F32)
            scr = sb.tile([SH, N], F32)  # dead store of the fused STT
            scra = sb.tile([SH, N], BF16)  # dead store of the C*s2i pass
            scrs = sb.tile([SH, N], BF16)  # dead store of the Sign pass
            red = sb.tile([SH, 4], F32)
            ones_col = sb.tile([SH, 1], F32)
            out_sb = sb.tile([1, 4], F32)

            d_ps = ps.tile([B, N], F32)  # 2 banks
            fin_ps = ps.tile([1, 4], F32)

            # Constants: consumer matmuls pair DVE-produced operands with
            # these, so they are born on DVE to keep matmul waits cheap.
            nc.vector.memset(nh, -0.5)
            nc.vector.memset(ones_col, 1.0)

            # Loads, all on the SP queue: split queues measured slower (the
            # 16 DMA engines are shared and the queues contend). Tiny PT
            # first (S2_i path), then P (heads the compute path), then C.
            nc.sync.dma_start(out=pt_sb, in_=pt_d[:, :])
            nc.sync.dma_start(out=p_sb, in_=p_d[:, :])
            nc.sync.dma_start(out=c_sb, in_=c_d[:, :])

            # psq = P*P in halves so the S-matmuls can start earlier
            nc.vector.tensor_mul(psq[:, 0:HALF], p_sb[:, 0:HALF], p_sb[:, 0:HALF])
            nc.vector.tensor_mul(psq[:, HALF:N], p_sb[:, HALF:N], p_sb[:, HALF:N])

            # red[:,3] = sum_j (C > 0)  — n_pos partials, on DVE (bf16 rate)
            nc.vector.tensor_scalar(
                scrs,
                c_sb,
                0.0,
                0.0,
                op0=mybir.AluOpType.is_gt,
                op1=mybir.AluOpType.add,
                accum_out=red[:, 3:4],
            )
            # s2i[i] = sum_b PT[i,b]^2 = S2 for the local rows, as a column
            nc.scalar.activation(
                ptsq, pt_sb, mybir.ActivationFunctionType.Square, accum_out=s2i
            )
            # red[:,2] = sum_j C * S2_i   (scale is a per-partition column)
            nc.scalar.activation(
                scra,
                c_sb,
                mybir.ActivationFunctionType.Copy,
                scale=s2i[:, 0:1],
                accum_out=red[:, 2:3],
            )

            # d = G - S2_j/2 accumulated per PSUM bank
            for h in range(2):
                js = slice(HALF * h, HALF * (h + 1))
                nc.tensor.matmul(
                    d_ps[:, js], p_sb[:, 0:SH], p_sb[:, js], start=True, stop=False
                )
                nc.tensor.matmul(
                    d_ps[:, js], nh, psq[:, js], start=False, stop=True
                )

            # red[:,h] = sum_j (C * -2) * d  (fused multiply+accumulate)
            for h in range(2):
                js = slice(HALF * h, HALF * (h + 1))
                nc.vector.scalar_tensor_tensor(
                    scr[:, js],
                    c_sb[:, js],
                    -2.0,
                    d_ps[:, js],
                    op0=mybir.AluOpType.mult,
                    op1=mybir.AluOpType.mult,
                    accum_out=red[:, h : h + 1],
                )

            # partition-reduce the four partial columns -> [1,4] on the PE
            # (idle by now; ~0.1us vs ~0.7us+hops for a gpsimd reduce)
            nc.tensor.matmul(fin_ps, ones_col, red, start=True, stop=True)
            nc.scalar.copy(out_sb, fin_ps)

            nc.sync.dma_start(out=o_d[:, :], in_=out_sb)

    _split_multi_waits(nc)
    return nc


def _split_multi_waits(nc: bass.Bass):
    """This walrus build accepts only ONE sync-wait per instruction
    ("Too many sync wait commands"). Tile's kernel-tail drain carries one
    wait per live semaphore; peel the extras onto same-engine NOPs that
    each stall on a single semaphore — semantically identical."""
    for bb in nc.main_func.blocks:
        insts = bb.instructions
        i = 0
        while i < len(insts):
            ins = insts[i]
            si = getattr(ins, "sync_info", None)
            if si is not None and si.on_wait is not None and len(si.on_wait) > 1:
                waits = list(si.on_wait)
                nops = []
                for j, w in enumerate(waits[:-1]):
                    nop = mybir.InstNoOp(
                        name=f"{ins.name}-wsplit{j}",
                        sync_info=mybir.SyncInfo(on_wait=[w], on_update=[]),
                        bass_nofuse=True,
                        engine=ins.engine,
                    )
                    nc.register_instruction(nop, overwrite=True)
                    nops.append(nop)
                si.on_wait = [waits[-1]]
                insts[i:i] = nops
                i += len(nops)
            i += 1


_NC = None


def _get_nc() -> bass.Bass:
    global _NC
    if _NC is None:
        _NC = build_bass()
    return _NC


def make_in_maps(probs: np.ndarray, co_matrix: np.ndarray):
    probs = np.asarray(probs, dtype=np.float32)
    co_matrix = np.asarray(co_matrix, dtype=np.float32)
    in_maps = []
    for k in range(NCORES):
        shift = -SH * k
        p_r = np.roll(probs, shift, axis=1)
        c_r = np.roll(co_matrix[SH * k : SH * (k + 1), :], shift, axis=1)
        in_maps.append(
            {
                "probs_r": np.ascontiguousarray(p_r.astype(BF16NP)),
                "pt_r": np.ascontiguousarray(p_r[:, 0:SH].T.astype(BF16NP)),
                "co_r": np.ascontiguousarray(c_r.astype(BF16NP)),
            }
        )
    return in_maps


def finish(outs: np.ndarray) -> np.ndarray:
    """outs: [NCORES, 1, 4] = per-core (stt0, stt1, C*S2_i, n_pos)."""
    o = outs.astype(np.float64)
    total = np.float32(o[:, 0, 0:3].sum())
    npos = np.float32(o[:, 0, 3].sum())
    loss = (total / np.float32(B)) / (npos + np.float32(1e-8))
    return np.array(loss, dtype=np.float32)


def kernel(probs: np.ndarray, co_matrix: np.ndarray) -> np.ndarray:
    nc = _get_nc()
    in_maps = make_in_maps(probs, co_matrix)
    res = run_bass_kernel_spmd(nc, in_maps, list(range(NCORES)))
    outs = np.stack([r["out"] for r in res.results])
    return finish(outs)


# revision 12
# speedup vs baseline: 1.2311x; 1.2311x over previous
"""Trainium2 Bass kernel for the KB criterion loss.

Math
----
reference:
    diff[b,i,j] = probs[b,j] - probs[b,i]
    loss = sum_ij mean_b (diff^2 * C[i,j]) / (n_pos + 1e-8),  n_pos = count(C > 0)

Expanding the square removes the [B,N,N] intermediate entirely:

    sum_b (P[b,i] - P[b,j])^2 = S2_i + S2_j - 2*G_ij
        with S2_j = sum_b P[b,j]^2   and   G = P^T P  (Gram matrix)

so   total = sum_ij C_ij * (S2_i + S2_j - 2 G_ij)
     loss  = (total / B) / (n_pos + 1e-8)

Sharding (8 cores)
------------------
Shard C by rows: core k owns rows S_k = [128k, 128k+128). P is replicated.
Each core moves ~0.5MB (bf16 P + bf16 C shard) vs 4MB of C with
batch-parallel sharding. Inputs are column-rolled by 128k so every core
runs the same program with its own row block mapped to local columns
[0:128). The transposed local P block (pure layout prep, like the roll)
ships as a third input so S2_i is available as a per-partition column.

Per-core pipeline (engines in parallel; all operands bf16, accum fp32):
  DVE : psq = P*P (two 512-col halves)
  PE  : per 512-col PSUM bank: d = P_Sk^T @ P  (Gram block, K=128 b)
                              + (-1/2 const)^T @ psq  (= -S2_j/2 bcast)
  ACT : s2i = rowsum(PT_Sk^2)  (Square + accum)  -> S2_i column
  DVE : (C * -2) * d  summed per partition  -> red[:,0:2]   (fused STT)
  ACT : C * s2i  summed per partition       -> red[:,2]  (Copy, scale AP)
  ACT : Sign(C)  summed per partition       -> red[:,3]  (n_pos count)
  Pool: partition-reduce red [128,4] -> [1,4], DMA out.

Host sums the 8 partial quadruples (the scalar all-reduce) and finishes
the division. bf16 operand rounding is well inside the 2e-2 tolerance
(errors are random over 2^20 summed terms; observed ~1e-3).
"""

import ml_dtypes
import numpy as np

import concourse.bass as bass
import concourse.tile as tile
from concourse import mybir
from concourse.bass_utils import run_bass_kernel_spmd

B = 128
N = 1024
NCORES = 8
SH = N // NCORES  # 128 rows of C per core
F32 = mybir.dt.float32
BF16 = mybir.dt.bfloat16
HALF = 512  # PSUM bank width in fp32
BF16NP = ml_dtypes.bfloat16


def build_bass() -> bass.Bass:
    nc = bass.Bass()
    p_d = nc.dram_tensor("probs_r", [B, N], BF16, kind="ExternalInput")
    pt_d = nc.dram_tensor("pt_r", [SH, B], BF16, kind="ExternalInput")
    c_d = nc.dram_tensor("co_r", [SH, N], BF16, kind="ExternalInput")
    o_d = nc.dram_tensor("out", [1, 4], F32, kind="ExternalOutput")

    with tile.TileContext(nc) as tc:
        with (
            tc.tile_pool(name="sb", bufs=1) as sb,
            tc.tile_pool(name="ps", bufs=1, space="PSUM") as ps,
        ):
            p_sb = sb.tile([B, N], BF16)
            pt_sb = sb.tile([SH, B], BF16)
            c_sb = sb.tile([SH, N], BF16)
            psq = sb.tile([B, N], BF16)
            nh = sb.tile([B, SH], BF16)  # const -1/2, lhsT of the S2_j matmul
            ptsq = sb.tile([SH, B], BF16)  # dead store of the Square pass
            s2i = sb.tile([SH, 1], F32)
            scr = sb.tile([SH, N], F32)  # dead store of the fused STT
            scra = sb.tile([SH, N], BF16)  # dead store of the C*s2i pass
            scrs = sb.tile([SH, N], BF16)  # dead store of the Sign pass
            red = sb.tile([SH, 4], F32)
            ones_col = sb.tile([SH, 1], F32)
            out_sb = sb.tile([1, 4], F32)

            d_ps = ps.tile([B, N], F32)  # 2 banks
            fin_ps = ps.tile([1, 4], F32)

            # Constants: consumer matmuls pair DVE-produced operands with
            # these, so they are born on DVE to keep matmul waits cheap.
            nc.vector.memset(nh, -0.5)
            nc.vector.memset(ones_col, 1.0)

            # Loads, all on the SP queue: split queues measured slower (the
            # 16 DMA engines are shared and the queues contend). Tiny PT
            # first (S2_i path), then P (heads the compute path), then C.
            nc.sync.dma_start(out=pt_sb, in_=pt_d[:, :])
            nc.sync.dma_start(out=p_sb, in_=p_d[:, :])
            nc.sync.dma_start(out=c_sb, in_=c_d[:, :])

            # psq = P*P in halves so the S-matmuls can start earlier
            nc.vector.tensor_mul(psq[:, 0:HALF], p_sb[:, 0:HALF], p_sb[:, 0:HALF])
            nc.vector.tensor_mul(psq[:, HALF:N], p_sb[:, HALF:N], p_sb[:, HALF:N])

            # s2i[i] = sum_b PT[i,b]^2 = S2 for the local rows, as a column
            nc.scalar.activation(
                ptsq, pt_sb, mybir.ActivationFunctionType.Square, accum_out=s2i
            )
            # red[:,2] = sum_j C * S2_i   (scale is a per-partition column)
            nc.scalar.activation(
                scra,
                c_sb,
                mybir.ActivationFunctionType.Copy,
                scale=s2i[:, 0:1],
                accum_out=red[:, 2:3],
            )

            # d = G - S2_j/2 accumulated per PSUM bank
            for h in range(2):
                js = slice(HALF * h, HALF * (h + 1))
                nc.tensor.matmul(
                    d_ps[:, js], p_sb[:, 0:SH], p_sb[:, js], start=True, stop=False
                )
                nc.tensor.matmul(
                    d_ps[:, js], nh, psq[:, js], start=False, stop=True
                )

            # red[:,h] = sum_j (C * -2) * d  (fused multiply+accumulate)
            for h in range(2):
                js = slice(HALF * h, HALF * (h + 1))
                nc.vector.scalar_tensor_tensor(
                    scr[:, js],
                    c_sb[:, js],
                    -2.0,
                    d_ps[:, js],
                    op0=mybir.AluOpType.mult,
                    op1=mybir.AluOpType.mult,
                    accum_out=red[:, h : h + 1],
                )

            # red[:,3] = sum_j sign(C) — n_pos partials (C >= 0 always);
            # emitted after Square/copy so those run first on ACT
            nc.scalar.activation(
                scrs, c_sb, mybir.ActivationFunctionType.Sign, accum_out=red[:, 3:4]
            )

            # partition-reduce the four partial columns -> [1,4] on the PE
            # (idle by now; ~0.1us vs ~0.7us+hops for a gpsimd reduce)
            nc.tensor.matmul(fin_ps, ones_col, red, start=True, stop=True)
            nc.scalar.copy(out_sb, fin_ps)

            nc.sync.dma_start(out=o_d[:, :], in_=out_sb)

    _split_multi_waits(nc)
    return nc


def _split_multi_waits(nc: bass.Bass):
    """This walrus build accepts only ONE sync-wait per instruction
    ("Too many sync wait commands"). Tile's kernel-tail drain carries one
    wait per live semaphore; peel the extras onto same-engine NOPs that
    each stall on a single semaphore — semantically identical."""
    for bb in nc.main_func.blocks:
        insts = bb.instructions
        i = 0
        while i < len(insts):
            ins = insts[i]
            si = getattr(ins, "sync_info", None)
            if si is not None and si.on_wait is not None and len(si.on_wait) > 1:
                waits = list(si.on_wait)
                nops = []
                for j, w in enumerate(waits[:-1]):
                    nop = mybir.InstNoOp(
                        name=f"{ins.name}-wsplit{j}",
                        sync_info=mybir.SyncInfo(on_wait=[w], on_update=[]),
                        bass_nofuse=True,
                        engine=ins.engine,
                    )
                    nc.register_instruction(nop, overwrite=True)
                    nops.append(nop)
                si.on_wait = [waits[-1]]
                insts[i:i] = nops
                i += len(nops)
            i += 1


_NC = None


def _get_nc() -> bass.Bass:
    global _NC
    if _NC is None:
        _NC = build_bass()
    return _NC


def make_in_maps(probs: np.ndarray, co_matrix: np.ndarray):
    probs = np.asarray(probs, dtype=np.float32)
    co_matrix = np.asarray(co_matrix, dtype=np.float32)
    in_maps = []
    for k in range(NCORES):
        shift = -SH * k
        p_r = np.roll(probs, shift, axis=1)
        c_r = np.roll(co_matrix[SH * k : SH * (k + 1), :], shift, axis=1)
        in_maps.append(
            {
                "probs_r": np.ascontiguousarray(p_r.astype(BF16NP)),
                "pt_r": np.ascontiguousarray(p_r[:, 0:SH].T.astype(BF16NP)),
                "co_r": np.ascontiguousarray(c_r.astype(BF16NP)),
            }
        )
    return in_maps


def finish(outs: np.ndarray) -> np.ndarray:
    """outs: [NCORES, 1, 4] = per-core (stt0, stt1, C*S2_i, n_pos)."""
    o = outs.astype(np.float64)
    total = np.float32(o[:, 0, 0:3].sum())
    npos = np.float32(o[:, 0, 3].sum())
    loss = (total / np.float32(B)) / (npos + np.float32(1e-8))
    return np.array(loss, dtype=np.float32)


def kernel(probs: np.ndarray, co_matrix: np.ndarray) -> np.ndarray:
    nc = _get_nc()
    in_maps = make_in_maps(probs, co_matrix)
    res = run_bass_kernel_spmd(nc, in_maps, list(range(NCORES)))
    outs = np.stack([r["out"] for r in res.results])
    return finish(outs)


# revision 17
# speedup vs baseline: 1.2633x; 1.0262x over previous
"""Trainium2 Bass kernel for the KB criterion loss.

Math
----
reference:
    diff[b,i,j] = probs[b,j] - probs[b,i]
    loss = sum_ij mean_b (diff^2 * C[i,j]) / (n_pos + 1e-8),  n_pos = count(C > 0)

Expanding the square removes the [B,N,N] intermediate entirely:

    sum_b (P[b,i] - P[b,j])^2 = S2_i + S2_j - 2*G_ij
        with S2_j = sum_b P[b,j]^2   and   G = P^T P  (Gram matrix)

so   total = sum_ij C_ij * (S2_i + S2_j - 2 G_ij)
     loss  = (total / B) / (n_pos + 1e-8)

Sharding (8 cores)
------------------
Shard C by rows: core k owns rows S_k = [128k, 128k+128). P is replicated.
Each core moves ~0.5MB (bf16 P + bf16 C shard) vs 4MB of C with
batch-parallel sharding. Inputs are column-rolled by 128k so every core
runs the same program with its own row block mapped to local columns
[0:128). The transposed local P block (pure layout prep, like the roll)
ships as a third input so S2_i is available as a per-partition column.

Per-core pipeline (engines in parallel; all operands bf16, accum fp32):
  DVE : psq = P*P (two 512-col halves)
  PE  : per 512-col PSUM bank: d = P_Sk^T @ P  (Gram block, K=128 b)
                              + (-1/2 const)^T @ psq  (= -S2_j/2 bcast)
  ACT : s2i = rowsum(PT_Sk^2)  (Square + accum)  -> S2_i column
  DVE : (C * -2) * d  summed per partition  -> red[:,0:2]   (fused STT)
  ACT : C * s2i  summed per partition       -> red[:,2]  (Copy, scale AP)
  ACT : Sign(C)  summed per partition       -> red[:,3]  (n_pos count)
  Pool: partition-reduce red [128,4] -> [1,4], DMA out.

Host sums the 8 partial quadruples (the scalar all-reduce) and finishes
the division. bf16 operand rounding is well inside the 2e-2 tolerance
(errors are random over 2^20 summed terms; observed ~1e-3).
"""

import ml_dtypes
import numpy as np

import concourse.bass as bass
import concourse.tile as tile
from concourse import mybir
from concourse.bass_utils import run_bass_kernel_spmd

B = 128
N = 1024
NCORES = 8
SH = N // NCORES  # 128 rows of C per core
F32 = mybir.dt.float32
BF16 = mybir.dt.bfloat16
HALF = 512  # PSUM bank width in fp32
BF16NP = ml_dtypes.bfloat16


def build_bass() -> bass.Bass:
    nc = bass.Bass()
    p_d = nc.dram_tensor("probs_r", [B, N], BF16, kind="ExternalInput")
    pt_d = nc.dram_tensor("pt_r", [SH, B], BF16, kind="ExternalInput")
    c_d = nc.dram_tensor("co_r", [SH, N], BF16, kind="ExternalInput")
    o_d = nc.dram_tensor("out", [1, 4], F32, kind="ExternalOutput")

    with tile.TileContext(nc) as tc:
        with (
            tc.tile_pool(name="sb", bufs=1) as sb,
            tc.tile_pool(name="ps", bufs=1, space="PSUM") as ps,
        ):
            p_sb = sb.tile([B, N], BF16)
            pt_sb = sb.tile([SH, B], BF16)
            c_sb = sb.tile([SH, N], BF16)
            psq = sb.tile([B, N], BF16)
            nh = sb.tile([B, SH], BF16)  # const -1/2, lhsT of the S2_j matmul
            ptsq = sb.tile([SH, B], BF16)  # dead store of the Square pass
            s2i = sb.tile([SH, 1], F32)
            scr = sb.tile([SH, N], BF16)  # dead store of the fused STT
            scra = sb.tile([SH, N], BF16)  # dead store of the C*s2i pass
            scrs = sb.tile([SH, N], BF16)  # dead store of the Sign pass
            red = sb.tile([SH, 4], F32)
            ones_col = sb.tile([SH, 1], F32)
            out_sb = sb.tile([1, 4], F32)

            d_ps = ps.tile([B, N], F32)  # 2 banks
            fin_ps = ps.tile([1, 4], F32)

            # Constants: consumer matmuls pair DVE-produced operands with
            # these, so they are born on DVE to keep matmul waits cheap.
            nc.vector.memset(nh, -0.5)
            nc.vector.memset(ones_col, 1.0)

            # Loads, all on the SP queue: split queues measured slower (the
            # 16 DMA engines are shared and the queues contend). Tiny PT
            # first (S2_i path), then P (heads the compute path), then C.
            nc.sync.dma_start(out=pt_sb, in_=pt_d[:, :])
            nc.sync.dma_start(out=p_sb, in_=p_d[:, :])
            nc.sync.dma_start(out=c_sb, in_=c_d[:, :])

            # psq = P*P in halves so the S-matmuls can start earlier
            nc.vector.tensor_mul(psq[:, 0:HALF], p_sb[:, 0:HALF], p_sb[:, 0:HALF])
            nc.vector.tensor_mul(psq[:, HALF:N], p_sb[:, HALF:N], p_sb[:, HALF:N])

            # s2i[i] = sum_b PT[i,b]^2 = S2 for the local rows, as a column
            nc.scalar.activation(
                ptsq, pt_sb, mybir.ActivationFunctionType.Square, accum_out=s2i
            )
            # red[:,2] = sum_j C * S2_i   (scale is a per-partition column)
            nc.scalar.activation(
                scra,
                c_sb,
                mybir.ActivationFunctionType.Copy,
                scale=s2i[:, 0:1],
                accum_out=red[:, 2:3],
            )

            # d = G - S2_j/2 accumulated per PSUM bank
            for h in range(2):
                js = slice(HALF * h, HALF * (h + 1))
                nc.tensor.matmul(
                    d_ps[:, js], p_sb[:, 0:SH], p_sb[:, js], start=True, stop=False
                )
                nc.tensor.matmul(
                    d_ps[:, js], nh, psq[:, js], start=False, stop=True
                )

            # red[:,h] = sum_j (C * -2) * d  (fused multiply+accumulate)
            for h in range(2):
                js = slice(HALF * h, HALF * (h + 1))
                nc.vector.scalar_tensor_tensor(
                    scr[:, js],
                    c_sb[:, js],
                    -2.0,
                    d_ps[:, js],
                    op0=mybir.AluOpType.mult,
                    op1=mybir.AluOpType.mult,
                    accum_out=red[:, h : h + 1],
                )

            # red[:,3] = sum_j sign(C) — n_pos partials (C >= 0 always);
            # emitted after Square/copy so those run first on ACT
            nc.scalar.activation(
                scrs, c_sb, mybir.ActivationFunctionType.Sign, accum_out=red[:, 3:4]
            )

            # partition-reduce the four partial columns -> [1,4] on the PE
            # (idle by now; ~0.1us vs ~0.7us+hops for a gpsimd reduce)
            nc.tensor.matmul(fin_ps, ones_col, red, start=True, stop=True)
            nc.scalar.copy(out_sb, fin_ps)

            nc.sync.dma_start(out=o_d[:, :], in_=out_sb)

    _split_multi_waits(nc)
    return nc


def _split_multi_waits(nc: bass.Bass):
    """This walrus build accepts only ONE sync-wait per instruction
    ("Too many sync wait commands"). Tile's kernel-tail drain carries one
    wait per live semaphore; peel the extras onto same-engine NOPs that
    each stall on a single semaphore — semantically identical."""
    for bb in nc.main_func.blocks:
        insts = bb.instructions
        i = 0
        while i < len(insts):
            ins = insts[i]
            si = getattr(ins, "sync_info", None)
            if si is not None and si.on_wait is not None and len(si.on_wait) > 1:
                waits = list(si.on_wait)
                nops = []
                for j, w in enumerate(waits[:-1]):
                    nop = mybir.InstNoOp(
                        name=f"{ins.name}-wsplit{j}",
                        sync_info=mybir.SyncInfo(on_wait=[w], on_update=[]),
                        bass_nofuse=True,
                        engine=ins.engine,
                    )
                    nc.register_instruction(nop, overwrite=True)
                    nops.append(nop)
                si.on_wait = [waits[-1]]
                insts[i:i] = nops
                i += len(nops)
            i += 1


_NC = None


def _get_nc() -> bass.Bass:
    global _NC
    if _NC is None:
        _NC = build_bass()
    return _NC


def make_in_maps(probs: np.ndarray, co_matrix: np.ndarray):
    probs = np.asarray(probs, dtype=np.float32)
    co_matrix = np.asarray(co_matrix, dtype=np.float32)
    in_maps = []
    for k in range(NCORES):
        shift = -SH * k
        p_r = np.roll(probs, shift, axis=1)
        c_r = np.roll(co_matrix[SH * k : SH * (k + 1), :], shift, axis=1)
        in_maps.append(
            {
                "probs_r": np.ascontiguousarray(p_r.astype(BF16NP)),
                "pt_r": np.ascontiguousarray(p_r[:, 0:SH].T.astype(BF16NP)),
                "co_r": np.ascontiguousarray(c_r.astype(BF16NP)),
            }
        )
    return in_maps


def finish(outs: np.ndarray) -> np.ndarray:
    """outs: [NCORES, 1, 4] = per-core (stt0, stt1, C*S2_i, n_pos)."""
    o = outs.astype(np.float64)
    total = np.float32(o[:, 0, 0:3].sum())
    npos = np.float32(o[:, 0, 3].sum())
    loss = (total / np.float32(B)) / (npos + np.float32(1e-8))
    return np.array(loss, dtype=np.float32)


def kernel(probs: np.ndarray, co_matrix: np.ndarray) -> np.ndarray:
    nc = _get_nc()
    in_maps = make_in_maps(probs, co_matrix)
    res = run_bass_kernel_spmd(nc, in_maps, list(range(NCORES)))
    outs = np.stack([r["out"] for r in res.results])
    return finish(outs)
